# revision 37
# baseline (speedup 1.0000x reference)
"""GCNConv (x @ W^T then edge-weighted SpMM segment-sum) on 8 Trainium2 cores.

Strategy (self-contained; shapes hardcoded for the graded problem):
  - Reformulate: out = A @ (x @ W^T) = (A @ x) @ W^T. The sparse gather/
    segment-sum runs on packed raw features x [N, B*F] (fp16), then a single
    dense W^T projection per row tile.
  - Shard output rows (nodes) contiguously across the 8 cores, balanced by
    edge count. Each core owns its rows' edges; x is replicated so gathers
    are core-local.
  - Edges are sorted by destination row and packed into "groups": each group
    covers <= 128 consecutive rows and a fixed budget of edge slots. Edge
    messages are fetched with dma_gather (SWDGE bulk gather, int16 indices);
    since N > 32767, x is split into even/odd row half-tables so indices
    (col // 2) fit in int16. Within a group, even-col edges and odd-col
    edges fill separate fixed-size chunk arrays, zero-padded.
  - Segment sum via TensorE: for each 128-edge chunk, an S matrix
    [128 edges, 128 rows] with S[e, lrow[e]] += val[e] (precomputed on the
    host, streamed in as fp16) is the stationary operand; PSUM accumulates
    sum_e val[e] * x[col[e]] per local row.
  - Per group: PSUM acc [128 rows, B*F] -> fp16 -> PE transpose per batch ->
    one W^T matmul over all batches -> y[g] = [F_out, B*128 rows] fp32 ->
    DRAM. Host scatters group tiles back to [B, N, F_out].
"""
from contextlib import ExitStack

import numpy as np

# ---- problem constants (hardcoded per contract) ----
B, N, E, F = 4, 50000, 800000, 128
NCORES = 8
P = 128
D = B * F          # 512 packed features per node
CE = CO = 8        # gather chunks per parity side per group
GM = 1             # groups per dma_gather call. Merged calls (GM=2, even at 113
                   # ring descriptors) proved unreliable on HW at full scale --
                   # keep one 1024-idx gather per parity side per group.
SLOT_E, SLOT_O = CE * P, CO * P
GBUFS = 7          # gather tile_pool bufs (must match _build_program)
NSH_E = (N + 1) // 2
NSH_O = N // 2


def _host_prep(x, W, edge_row, edge_col, edge_vals):
    """Sort/shard/pack edges; returns (shared arrays, per-core arrays, assembly)."""
    edge_row = np.asarray(edge_row).astype(np.int64)
    edge_col = np.asarray(edge_col).astype(np.int64)
    edge_vals = np.asarray(edge_vals).astype(np.float32)

    xp = np.ascontiguousarray(np.transpose(np.asarray(x), (1, 0, 2))).reshape(N, D)
    xp16 = xp.astype(np.float16)
    xE = np.ascontiguousarray(xp16[0::2])   # [NSH_E, D]
    xO = np.ascontiguousarray(xp16[1::2])   # [NSH_O, D]
    wt = np.asarray(W).T.astype(np.float16)  # [F_in, F_out]
    ident = np.eye(P, dtype=np.float16)

    order = np.argsort(edge_row, kind="stable")
    r_s = edge_row[order]
    c_s = edge_col[order]
    v_s = edge_vals[order]
    counts = np.bincount(r_s, minlength=N)
    cum = np.zeros(N + 1, np.int64)
    np.cumsum(counts, out=cum[1:])
    ne_total = r_s.shape[0]
    even_pref = np.zeros(ne_total + 1, np.int64)
    np.cumsum((c_s % 2) == 0, out=even_pref[1:])
    odd_pref = np.arange(ne_total + 1) - even_pref
    evenc_at_row = even_pref[cum]            # [N+1]
    oddc_at_row = cum - evenc_at_row

    # core row boundaries balanced by edge count
    bounds = [0]
    for c in range(1, NCORES):
        bounds.append(int(np.searchsorted(cum, c * ne_total // NCORES)))
    bounds.append(N)

    per_core = []
    for c in range(NCORES):
        rs0, rs1 = bounds[c], bounds[c + 1]
        groups = []  # (row_base, nrows, edge_lo, edge_hi)
        r = rs0
        while r < rs1:
            re_e = int(np.searchsorted(evenc_at_row, evenc_at_row[r] + SLOT_E,
                                       side="right")) - 1
            re_o = int(np.searchsorted(oddc_at_row, oddc_at_row[r] + SLOT_O,
                                       side="right")) - 1
            re = min(re_e, re_o, r + P, rs1)
            if re > r:
                groups.append((r, re - r, int(cum[r]), int(cum[re])))
                r = re
            else:
                # single row exceeds the per-group slot caps: split its edge
                # list across several single-row groups (host assembly sums)
                e0, e1 = int(cum[r]), int(cum[r + 1])
                while e0 < e1:
                    e2a = int(np.searchsorted(
                        even_pref, even_pref[e0] + SLOT_E, side="right")) - 1
                    e2b = int(np.searchsorted(
                        odd_pref, odd_pref[e0] + SLOT_O, side="right")) - 1
                    e2 = min(e2a, e2b, e1)
                    assert e2 > e0
                    groups.append((r, 1, e0, e2))
                    e0 = e2
                r += 1
        per_core.append(groups)

    NG = max(len(g) for g in per_core)
    NG = ((NG + GM - 1) // GM) * GM   # pad so gather calls merge GM groups

    core_inputs = []
    assembly = []
    for c in range(NCORES):
        groups = per_core[c]
        CH = CE + CO
        # Slots >= the per-group count hold -1: the gather ucode skips
        # trailing negative indices entirely (no descriptors, no bytes).
        # Counts ride in cnt[] and are loaded into a GPR per gather at
        # runtime (they differ per core under the shared SPMD program).
        # Each count is >= 16 so every one of the 16 DMA engines gets at
        # least one descriptor (its completion-sem bump).
        idxE = np.full((NG, SLOT_E), -1, np.int16)
        idxO = np.full((NG, SLOT_O), -1, np.int16)
        cnt = np.zeros(2 * NG, np.int32)
        # S is built on-chip: stream only (lrow, val) per edge slot.
        # lrow_all[p, g*CH + j] = local row of edge slot j*128+p of group g
        # (-1 for empty slots so the on-chip iota compare yields 0).
        lrow_all = np.full((P, NG * CH), -1.0, np.float16)
        sval_all = np.zeros((P, NG * CH), np.float16)
        bases, nrows = [], []
        for g, (base, nr, elo, ehi) in enumerate(groups):
            cols = c_s[elo:ehi]
            vals_g = v_s[elo:ehi]
            lrow_g = (r_s[elo:ehi] - base).astype(np.int64)
            em = (cols % 2) == 0
            ne, no = int(em.sum()), int((~em).sum())
            assert ne <= SLOT_E and no <= SLOT_O
            idxE[g, :ne] = (cols[em] >> 1).astype(np.int16)
            idxO[g, :no] = (cols[~em] >> 1).astype(np.int16)
            if g < GBUFS:
                # First ring of gather buffers: gather full slots so no
                # buffer ever holds uninitialized SBUF (0 * NaN = NaN in the
                # PSUM accumulate). Later groups recycle these buffers, so
                # their skipped tails hold stale-but-finite x values.
                idxE[g, ne:] = 0
                idxO[g, no:] = 0
                cnt[2 * g] = SLOT_E
                cnt[2 * g + 1] = SLOT_O
            else:
                if ne < 16:
                    idxE[g, ne:16] = 0
                if no < 16:
                    idxO[g, no:16] = 0
                cnt[2 * g] = max(ne, 16)
                cnt[2 * g + 1] = max(no, 16)
            slot = np.empty(ne + no, np.int64)
            slot[:ne] = np.arange(ne)
            slot[ne:] = SLOT_E + np.arange(no)
            lr_all = np.concatenate([lrow_g[em], lrow_g[~em]])
            v_all = np.concatenate([vals_g[em], vals_g[~em]])
            lrow_all[slot % P, g * CH + slot // P] = lr_all
            sval_all[slot % P, g * CH + slot // P] = v_all
            bases.append(base)
            nrows.append(nr)
        for g in range(len(groups), NG):  # padding groups: 16 junk slots
            idxE[g, :16] = 0
            idxO[g, :16] = 0
            cnt[2 * g] = cnt[2 * g + 1] = 16
        # idx slot i -> [i % 16, i // 16], replicated across 8 Q7 partitions
        idxE_t = np.ascontiguousarray(
            np.tile(idxE.reshape(NG, SLOT_E // 16, 16).transpose(2, 0, 1)
                    .reshape(1, 16, -1), (8, 1, 1)).reshape(P, -1))
        idxO_t = np.ascontiguousarray(
            np.tile(idxO.reshape(NG, SLOT_O // 16, 16).transpose(2, 0, 1)
                    .reshape(1, 16, -1), (8, 1, 1)).reshape(P, -1))
        core_inputs.append(dict(idxE=idxE_t, idxO=idxO_t,
                                lrow=lrow_all, sval=sval_all,
                                cnt=cnt.reshape(1, -1)))
        assembly.append((bases, nrows))

    # iota[p, r, j] = r  (materialized so every tensor_tensor operand keeps a
    # stride-1 last dim -> DVE 2x mode)
    CH = CE + CO
    iota = np.ascontiguousarray(np.broadcast_to(
        np.arange(P, dtype=np.float16)[None, :, None], (P, P, CH)))
    shared = dict(xE=xE, xO=xO, wt=wt, ident=ident, iota=iota.reshape(P, -1))
    return shared, core_inputs, assembly, NG


def _build_program(nc, NG):
    import concourse.tile as tile
    from concourse import mybir

    f16 = mybir.dt.float16
    f32 = mybir.dt.float32
    i16 = mybir.dt.int16
    i32 = mybir.dt.int32
    CH = CE + CO

    xE_d = nc.dram_tensor("xE", [NSH_E, D], f16, kind="ExternalInput")
    xO_d = nc.dram_tensor("xO", [NSH_O, D], f16, kind="ExternalInput")
    idxE_d = nc.dram_tensor("idxE", [P, NG * (SLOT_E // 16)], i16,
                            kind="ExternalInput")
    idxO_d = nc.dram_tensor("idxO", [P, NG * (SLOT_O // 16)], i16,
                            kind="ExternalInput")
    cnt_d = nc.dram_tensor("cnt", [1, 2 * NG], i32, kind="ExternalInput")
    lrow_d = nc.dram_tensor("lrow", [P, NG * CH], f16, kind="ExternalInput")
    sval_d = nc.dram_tensor("sval", [P, NG * CH], f16, kind="ExternalInput")
    iota_d = nc.dram_tensor("iota", [P, P * CH], f16, kind="ExternalInput")
    wt_d = nc.dram_tensor("wt", [P, P], f16, kind="ExternalInput")
    ident_d = nc.dram_tensor("ident", [P, P], f16, kind="ExternalInput")
    # y[g] = [F_out, B*P]: column b*P + r holds batch b, local row r
    y_d = nc.dram_tensor("y", [NG, P, B * P], f16, kind="ExternalOutput")

    with ExitStack() as ctx:
        tc = ctx.enter_context(tile.TileContext(nc))
        constp = ctx.enter_context(tc.tile_pool(name="const", bufs=1))
        gpE = ctx.enter_context(tc.tile_pool(name="gE", bufs=GBUFS))
        gpO = ctx.enter_context(tc.tile_pool(name="gO", bufs=GBUFS))
        sp = ctx.enter_context(tc.tile_pool(name="s", bufs=4))
        accp = ctx.enter_context(tc.tile_pool(name="acc", bufs=3, space="PSUM"))
        psum2 = ctx.enter_context(tc.tile_pool(name="ps2", bufs=2, space="PSUM"))
        postp = ctx.enter_context(tc.tile_pool(name="post", bufs=4))
        # y staging gets a deep ring: y-write DMAs can sit ~30us behind the
        # gather backlog in the engine FIFOs without stalling the scalar queue
        ysp = ctx.enter_context(tc.tile_pool(name="ys", bufs=12))

        wE = SLOT_E // 16
        wO = SLOT_O // 16
        # split idx loads so the first gathers only wait on a small prefix DMA
        PREG = min(4, NG)
        cnt_t = constp.tile([1, 2 * NG], i32, tag="cnt")
        nc.sync.dma_start(cnt_t[:], cnt_d[:])
        idxE_a = constp.tile([P, PREG * wE], i16, tag="idxEa")
        nc.sync.dma_start(idxE_a[:], idxE_d[:, :PREG * wE])
        idxO_a = constp.tile([P, PREG * wO], i16, tag="idxOa")
        nc.sync.dma_start(idxO_a[:], idxO_d[:, :PREG * wO])
        wt_t = constp.tile([P, P], f16)
        nc.sync.dma_start(wt_t[:], wt_d[:])
        ident = constp.tile([P, P], f16)
        nc.sync.dma_start(ident[:], ident_d[:])
        lrow_t = constp.tile([P, NG, CH], f16, tag="lrow")
        nc.sync.dma_start(lrow_t[:], lrow_d[:])
        sval_t = constp.tile([P, NG, CH], f16, tag="sval")
        nc.sync.dma_start(sval_t[:], sval_d[:])
        iota_t = constp.tile([P, P, CH], f16, tag="iota")
        nc.sync.dma_start(iota_t[:], iota_d[:])
        idxE_b = constp.tile([P, (NG - PREG) * wE], i16, tag="idxEb")
        nc.sync.dma_start(idxE_b[:], idxE_d[:, PREG * wE:])
        idxO_b = constp.tile([P, (NG - PREG) * wO], i16, tag="idxOb")
        nc.sync.dma_start(idxO_b[:], idxO_d[:, PREG * wO:])
        # Software-pipelined: group g's tail (PSUM->SBUF, transposes, W
        # projection, y write) issues AFTER group g+1's chunk matmuls so the
        # Tensor queue never head-of-line blocks on the scalar/DVE tail chain.
        pend = None  # (g, acc)

        def tail(g, acc):
            accS = postp.tile([P, D], f16, tag="accS")
            nc.scalar.copy(accS[:], acc[:])
            accTs = postp.tile([P, B * P], f16, tag="accTs")
            for b in range(B):
                accT = psum2.tile([P, P], f16, tag="accT")
                nc.tensor.transpose(accT[:], accS[:, b * P:(b + 1) * P],
                                    ident[:])
                nc.vector.tensor_copy(accTs[:, b * P:(b + 1) * P], accT[:])
            yb = psum2.tile([P, B * P], f32, tag="yb")
            nc.tensor.matmul(yb[:], lhsT=wt_t[:], rhs=accTs[:],
                             start=True, stop=True)
            ys = ysp.tile([P, B * P], f16, tag="ys")
            nc.scalar.copy(ys[:], yb[:])
            nc.sync.dma_start(y_d[g], ys[:])

        for gq in range(NG // GM):
            g0 = gq * GM
            if g0 < PREG:
                iE = idxE_a[:, g0 * wE:(g0 + GM) * wE]
                iO = idxO_a[:, g0 * wO:(g0 + GM) * wO]
            else:
                iE = idxE_b[:, (g0 - PREG) * wE:(g0 - PREG + GM) * wE]
                iO = idxO_b[:, (g0 - PREG) * wO:(g0 - PREG + GM) * wO]
            gtE = gpE.tile([P, GM * CE, D], f16)
            with nc.gpsimd.register() as rE:
                nc.gpsimd.reg_load(rE, cnt_t[0:1, 2 * g0:2 * g0 + 1])
                nc.gpsimd.dma_gather(
                    gtE[:], xE_d[:], iE,
                    num_idxs=GM * SLOT_E, num_idxs_reg=rE, elem_size=D,
                    queue_num=(2 * gq) % 4)
            gtO = gpO.tile([P, GM * CO, D], f16)
            with nc.gpsimd.register() as rO:
                nc.gpsimd.reg_load(rO, cnt_t[0:1, 2 * g0 + 1:2 * g0 + 2])
                nc.gpsimd.dma_gather(
                    gtO[:], xO_d[:], iO,
                    num_idxs=GM * SLOT_O, num_idxs_reg=rO, elem_size=D,
                    queue_num=(2 * gq + 1) % 4)
            for gi in range(GM):
                g = gq * GM + gi
                # Build S[p, r, j] = (r == lrow[p, g, j]) * sval[p, g, j] on
                # DVE. Layout keeps the broadcast on the middle dim so every
                # operand's last dim is stride-1 (DVE 2x mode); the matmul
                # reads lhsT strided as s_t[:, :, j].
                s_t = sp.tile([P, P, CH], f16)
                nc.vector.tensor_tensor(
                    out=s_t[:], in0=iota_t[:],
                    in1=lrow_t[:, g, None, :].broadcast_to([P, P, CH]),
                    op=mybir.AluOpType.is_equal)
                nc.vector.tensor_tensor(
                    out=s_t[:], in0=s_t[:],
                    in1=sval_t[:, g, None, :].broadcast_to([P, P, CH]),
                    op=mybir.AluOpType.mult)
                acc = accp.tile([P, D], f32)
                for j in range(CH):
                    rhs = (gtE[:, gi * CE + j, :] if j < CE
                           else gtO[:, gi * CO + j - CE, :])
                    nc.tensor.matmul(acc[:], lhsT=s_t[:, :, j],
                                     rhs=rhs,
                                     start=(j == 0), stop=(j == CH - 1))
                if pend is not None:
                    tail(*pend)
                pend = (g, acc)
        tail(*pend)
    return y_d


_PROFILE = False
_TRACE_DIR = None
_LAST_RESULT = None


def kernel(x, W, edge_row, edge_col, edge_vals):
    global _LAST_RESULT
    from concourse import bacc
    from concourse.bass_utils import run_bass_kernel_spmd

    shared, core_inputs, assembly, NG = _host_prep(
        x, W, edge_row, edge_col, edge_vals)

    nc = bacc.Bacc(num_swdge_queues=4)
    _build_program(nc, NG)
    nc.compile()

    in_maps = [dict(shared, **ci) for ci in core_inputs]
    res = run_bass_kernel_spmd(nc, in_maps, core_ids=list(range(NCORES)),
                               trace=_PROFILE, tmpdir=_TRACE_DIR)
    _LAST_RESULT = res

    out = np.zeros((B, N, F), np.float32)
    for c in range(NCORES):
        yc = res.results[c]["y"].astype(np.float32).reshape(-1, P, B, P)
        bases, nrows = assembly[c]
        for g, (base, nr) in enumerate(zip(bases, nrows)):
            out[:, base:base + nr, :] += yc[g, :, :, :nr].transpose(1, 2, 0)
    return out



# revision 38
# speedup vs baseline: 1.0074x; 1.0074x over previous
"""GCNConv (x @ W^T then edge-weighted SpMM segment-sum) on 8 Trainium2 cores.

Strategy (self-contained; shapes hardcoded for the graded problem):
  - Reformulate: out = A @ (x @ W^T) = (A @ x) @ W^T. The sparse gather/
    segment-sum runs on packed raw features x [N, B*F] (fp16), then a single
    dense W^T projection per row tile.
  - Shard output rows (nodes) contiguously across the 8 cores, balanced by
    edge count. Each core owns its rows' edges; x is replicated so gathers
    are core-local.
  - Edges are sorted by destination row and packed into "groups": each group
    covers <= 128 consecutive rows and a fixed budget of edge slots. Edge
    messages are fetched with dma_gather (SWDGE bulk gather, int16 indices);
    since N > 32767, x is split into even/odd row half-tables so indices
    (col // 2) fit in int16. Within a group, even-col edges and odd-col
    edges fill separate fixed-size chunk arrays, zero-padded.
  - Segment sum via TensorE: for each 128-edge chunk, an S matrix
    [128 edges, 128 rows] with S[e, lrow[e]] += val[e] (precomputed on the
    host, streamed in as fp16) is the stationary operand; PSUM accumulates
    sum_e val[e] * x[col[e]] per local row.
  - Per group: PSUM acc [128 rows, B*F] -> fp16 -> PE transpose per batch ->
    one W^T matmul over all batches -> y[g] = [F_out, B*128 rows] fp32 ->
    DRAM. Host scatters group tiles back to [B, N, F_out].
"""
from contextlib import ExitStack

import numpy as np

# ---- problem constants (hardcoded per contract) ----
B, N, E, F = 4, 50000, 800000, 128
NCORES = 8
P = 128
D = B * F          # 512 packed features per node
CE = CO = 8        # gather chunks per parity side per group
GM = 1             # groups per dma_gather call. Merged calls (GM=2, even at 113
                   # ring descriptors) proved unreliable on HW at full scale --
                   # keep one 1024-idx gather per parity side per group.
SLOT_E, SLOT_O = CE * P, CO * P
GBUFS = 7          # gather tile_pool bufs (must match _build_program)
NSH_E = (N + 1) // 2
NSH_O = N // 2


def _host_prep(x, W, edge_row, edge_col, edge_vals):
    """Sort/shard/pack edges; returns (shared arrays, per-core arrays, assembly)."""
    edge_row = np.asarray(edge_row).astype(np.int64)
    edge_col = np.asarray(edge_col).astype(np.int64)
    edge_vals = np.asarray(edge_vals).astype(np.float32)

    xp = np.ascontiguousarray(np.transpose(np.asarray(x), (1, 0, 2))).reshape(N, D)
    xp16 = xp.astype(np.float16)
    xE = np.ascontiguousarray(xp16[0::2])   # [NSH_E, D]
    xO = np.ascontiguousarray(xp16[1::2])   # [NSH_O, D]
    wt = np.asarray(W).T.astype(np.float16)  # [F_in, F_out]
    ident = np.eye(P, dtype=np.float16)

    order = np.argsort(edge_row, kind="stable")
    r_s = edge_row[order]
    c_s = edge_col[order]
    v_s = edge_vals[order]
    counts = np.bincount(r_s, minlength=N)
    cum = np.zeros(N + 1, np.int64)
    np.cumsum(counts, out=cum[1:])
    ne_total = r_s.shape[0]
    even_pref = np.zeros(ne_total + 1, np.int64)
    np.cumsum((c_s % 2) == 0, out=even_pref[1:])
    odd_pref = np.arange(ne_total + 1) - even_pref
    evenc_at_row = even_pref[cum]            # [N+1]
    oddc_at_row = cum - evenc_at_row

    # core row boundaries balanced by edge count
    bounds = [0]
    for c in range(1, NCORES):
        bounds.append(int(np.searchsorted(cum, c * ne_total // NCORES)))
    bounds.append(N)

    per_core = []
    for c in range(NCORES):
        rs0, rs1 = bounds[c], bounds[c + 1]
        groups = []  # (row_base, nrows, edge_lo, edge_hi)
        r = rs0
        while r < rs1:
            re_e = int(np.searchsorted(evenc_at_row, evenc_at_row[r] + SLOT_E,
                                       side="right")) - 1
            re_o = int(np.searchsorted(oddc_at_row, oddc_at_row[r] + SLOT_O,
                                       side="right")) - 1
            re = min(re_e, re_o, r + P, rs1)
            if re > r:
                groups.append((r, re - r, int(cum[r]), int(cum[re])))
                r = re
            else:
                # single row exceeds the per-group slot caps: split its edge
                # list across several single-row groups (host assembly sums)
                e0, e1 = int(cum[r]), int(cum[r + 1])
                while e0 < e1:
                    e2a = int(np.searchsorted(
                        even_pref, even_pref[e0] + SLOT_E, side="right")) - 1
                    e2b = int(np.searchsorted(
                        odd_pref, odd_pref[e0] + SLOT_O, side="right")) - 1
                    e2 = min(e2a, e2b, e1)
                    assert e2 > e0
                    groups.append((r, 1, e0, e2))
                    e0 = e2
                r += 1
        per_core.append(groups)

    NG = max(len(g) for g in per_core)
    NG = ((NG + GM - 1) // GM) * GM   # pad so gather calls merge GM groups

    core_inputs = []
    assembly = []
    for c in range(NCORES):
        groups = per_core[c]
        CH = CE + CO
        # Slots >= the per-group count hold -1: the gather ucode skips
        # trailing negative indices entirely (no descriptors, no bytes).
        # Counts ride in cnt[] and are loaded into a GPR per gather at
        # runtime (they differ per core under the shared SPMD program).
        # Each count is >= 16 so every one of the 16 DMA engines gets at
        # least one descriptor (its completion-sem bump).
        idxE = np.full((NG, SLOT_E), -1, np.int16)
        idxO = np.full((NG, SLOT_O), -1, np.int16)
        cnt = np.zeros(2 * NG, np.int32)
        # S is built on-chip: stream only (lrow, val) per edge slot.
        # lrow_all[p, g*CH + j] = local row of edge slot j*128+p of group g
        # (-1 for empty slots so the on-chip iota compare yields 0).
        lrow_all = np.full((P, NG * CH), -1.0, np.float16)
        sval_all = np.zeros((P, NG * CH), np.float16)
        bases, nrows = [], []
        for g, (base, nr, elo, ehi) in enumerate(groups):
            cols = c_s[elo:ehi]
            vals_g = v_s[elo:ehi]
            lrow_g = (r_s[elo:ehi] - base).astype(np.int64)
            em = (cols % 2) == 0
            ne, no = int(em.sum()), int((~em).sum())
            assert ne <= SLOT_E and no <= SLOT_O
            idxE[g, :ne] = (cols[em] >> 1).astype(np.int16)
            idxO[g, :no] = (cols[~em] >> 1).astype(np.int16)
            if g < GBUFS:
                # First ring of gather buffers: gather full slots so no
                # buffer ever holds uninitialized SBUF (0 * NaN = NaN in the
                # PSUM accumulate). Later groups recycle these buffers, so
                # their skipped tails hold stale-but-finite x values.
                idxE[g, ne:] = 0
                idxO[g, no:] = 0
                cnt[2 * g] = SLOT_E
                cnt[2 * g + 1] = SLOT_O
            else:
                if ne < 16:
                    idxE[g, ne:16] = 0
                if no < 16:
                    idxO[g, no:16] = 0
                cnt[2 * g] = max(ne, 16)
                cnt[2 * g + 1] = max(no, 16)
            slot = np.empty(ne + no, np.int64)
            slot[:ne] = np.arange(ne)
            slot[ne:] = SLOT_E + np.arange(no)
            lr_all = np.concatenate([lrow_g[em], lrow_g[~em]])
            v_all = np.concatenate([vals_g[em], vals_g[~em]])
            lrow_all[slot % P, g * CH + slot // P] = lr_all
            sval_all[slot % P, g * CH + slot // P] = v_all
            bases.append(base)
            nrows.append(nr)
        for g in range(len(groups), NG):  # padding groups: 16 junk slots
            idxE[g, :16] = 0
            idxO[g, :16] = 0
            cnt[2 * g] = cnt[2 * g + 1] = 16
        # idx slot i -> [i % 16, i // 16], replicated across 8 Q7 partitions
        idxE_t = np.ascontiguousarray(
            np.tile(idxE.reshape(NG, SLOT_E // 16, 16).transpose(2, 0, 1)
                    .reshape(1, 16, -1), (8, 1, 1)).reshape(P, -1))
        idxO_t = np.ascontiguousarray(
            np.tile(idxO.reshape(NG, SLOT_O // 16, 16).transpose(2, 0, 1)
                    .reshape(1, 16, -1), (8, 1, 1)).reshape(P, -1))
        core_inputs.append(dict(idxE=idxE_t, idxO=idxO_t,
                                lrow=lrow_all, sval=sval_all,
                                cnt=cnt.reshape(1, -1)))
        assembly.append((bases, nrows))

    # iota[p, r, j] = r  (materialized so every tensor_tensor operand keeps a
    # stride-1 last dim -> DVE 2x mode)
    CH = CE + CO
    iota = np.ascontiguousarray(np.broadcast_to(
        np.arange(P, dtype=np.float16)[None, :, None], (P, P, CH)))
    shared = dict(xE=xE, xO=xO, wt=wt, ident=ident, iota=iota.reshape(P, -1))
    return shared, core_inputs, assembly, NG


def _build_program(nc, NG):
    import concourse.tile as tile
    from concourse import mybir

    f16 = mybir.dt.float16
    f32 = mybir.dt.float32
    i16 = mybir.dt.int16
    i32 = mybir.dt.int32
    CH = CE + CO

    xE_d = nc.dram_tensor("xE", [NSH_E, D], f16, kind="ExternalInput")
    xO_d = nc.dram_tensor("xO", [NSH_O, D], f16, kind="ExternalInput")
    idxE_d = nc.dram_tensor("idxE", [P, NG * (SLOT_E // 16)], i16,
                            kind="ExternalInput")
    idxO_d = nc.dram_tensor("idxO", [P, NG * (SLOT_O // 16)], i16,
                            kind="ExternalInput")
    cnt_d = nc.dram_tensor("cnt", [1, 2 * NG], i32, kind="ExternalInput")
    lrow_d = nc.dram_tensor("lrow", [P, NG * CH], f16, kind="ExternalInput")
    sval_d = nc.dram_tensor("sval", [P, NG * CH], f16, kind="ExternalInput")
    iota_d = nc.dram_tensor("iota", [P, P * CH], f16, kind="ExternalInput")
    wt_d = nc.dram_tensor("wt", [P, P], f16, kind="ExternalInput")
    ident_d = nc.dram_tensor("ident", [P, P], f16, kind="ExternalInput")
    # y[g] = [F_out, B*P]: column b*P + r holds batch b, local row r
    y_d = nc.dram_tensor("y", [NG, P, B * P], f16, kind="ExternalOutput")

    with ExitStack() as ctx:
        tc = ctx.enter_context(tile.TileContext(nc))
        constp = ctx.enter_context(tc.tile_pool(name="const", bufs=1))
        gpE = ctx.enter_context(tc.tile_pool(name="gE", bufs=GBUFS))
        gpO = ctx.enter_context(tc.tile_pool(name="gO", bufs=GBUFS))
        sp = ctx.enter_context(tc.tile_pool(name="s", bufs=6))
        accp = ctx.enter_context(tc.tile_pool(name="acc", bufs=4, space="PSUM"))
        psum2 = ctx.enter_context(tc.tile_pool(name="ps2", bufs=2, space="PSUM"))
        postp = ctx.enter_context(tc.tile_pool(name="post", bufs=4))
        # y staging gets a deep ring: y-write DMAs can sit ~30us behind the
        # gather backlog in the engine FIFOs without stalling the scalar queue
        ysp = ctx.enter_context(tc.tile_pool(name="ys", bufs=12))

        wE = SLOT_E // 16
        wO = SLOT_O // 16
        # split idx loads so the first gathers only wait on a small prefix DMA
        PREG = min(4, NG)
        cnt_t = constp.tile([1, 2 * NG], i32, tag="cnt")
        nc.sync.dma_start(cnt_t[:], cnt_d[:])
        idxE_a = constp.tile([P, PREG * wE], i16, tag="idxEa")
        nc.sync.dma_start(idxE_a[:], idxE_d[:, :PREG * wE])
        idxO_a = constp.tile([P, PREG * wO], i16, tag="idxOa")
        nc.sync.dma_start(idxO_a[:], idxO_d[:, :PREG * wO])
        wt_t = constp.tile([P, P], f16)
        nc.sync.dma_start(wt_t[:], wt_d[:])
        ident = constp.tile([P, P], f16)
        nc.sync.dma_start(ident[:], ident_d[:])
        lrow_t = constp.tile([P, NG, CH], f16, tag="lrow")
        nc.sync.dma_start(lrow_t[:], lrow_d[:])
        sval_t = constp.tile([P, NG, CH], f16, tag="sval")
        nc.sync.dma_start(sval_t[:], sval_d[:])
        iota_t = constp.tile([P, P, CH], f16, tag="iota")
        nc.sync.dma_start(iota_t[:], iota_d[:])
        idxE_b = constp.tile([P, (NG - PREG) * wE], i16, tag="idxEb")
        nc.sync.dma_start(idxE_b[:], idxE_d[:, PREG * wE:])
        idxO_b = constp.tile([P, (NG - PREG) * wO], i16, tag="idxOb")
        nc.sync.dma_start(idxO_b[:], idxO_d[:, PREG * wO:])
        # Software-pipelined: group g's tail (PSUM->SBUF, transposes, W
        # projection, y write) issues AFTER group g+1's chunk matmuls so the
        # Tensor queue never head-of-line blocks on the scalar/DVE tail chain.
        pend = None  # (g, acc)

        def tail(g, acc):
            accS = postp.tile([P, D], f16, tag="accS")
            nc.scalar.copy(accS[:], acc[:])
            accTs = postp.tile([P, B * P], f16, tag="accTs")
            for b in range(B):
                accT = psum2.tile([P, P], f16, tag="accT")
                nc.tensor.transpose(accT[:], accS[:, b * P:(b + 1) * P],
                                    ident[:])
                nc.vector.tensor_copy(accTs[:, b * P:(b + 1) * P], accT[:])
            yb = psum2.tile([P, B * P], f32, tag="yb")
            nc.tensor.matmul(yb[:], lhsT=wt_t[:], rhs=accTs[:],
                             start=True, stop=True)
            ys = ysp.tile([P, B * P], f16, tag="ys")
            nc.scalar.copy(ys[:], yb[:])
            nc.sync.dma_start(y_d[g], ys[:])

        for gq in range(NG // GM):
            g0 = gq * GM
            if g0 < PREG:
                iE = idxE_a[:, g0 * wE:(g0 + GM) * wE]
                iO = idxO_a[:, g0 * wO:(g0 + GM) * wO]
            else:
                iE = idxE_b[:, (g0 - PREG) * wE:(g0 - PREG + GM) * wE]
                iO = idxO_b[:, (g0 - PREG) * wO:(g0 - PREG + GM) * wO]
            gtE = gpE.tile([P, GM * CE, D], f16)
            with nc.gpsimd.register() as rE:
                nc.gpsimd.reg_load(rE, cnt_t[0:1, 2 * g0:2 * g0 + 1])
                nc.gpsimd.dma_gather(
                    gtE[:], xE_d[:], iE,
                    num_idxs=GM * SLOT_E, num_idxs_reg=rE, elem_size=D,
                    queue_num=(2 * gq) % 4)
            gtO = gpO.tile([P, GM * CO, D], f16)
            with nc.gpsimd.register() as rO:
                nc.gpsimd.reg_load(rO, cnt_t[0:1, 2 * g0 + 1:2 * g0 + 2])
                nc.gpsimd.dma_gather(
                    gtO[:], xO_d[:], iO,
                    num_idxs=GM * SLOT_O, num_idxs_reg=rO, elem_size=D,
                    queue_num=(2 * gq + 1) % 4)
            for gi in range(GM):
                g = gq * GM + gi
                # Build S[p, r, j] = (r == lrow[p, g, j]) * sval[p, g, j] on
                # DVE. Layout keeps the broadcast on the middle dim so every
                # operand's last dim is stride-1 (DVE 2x mode); the matmul
                # reads lhsT strided as s_t[:, :, j].
                s_t = sp.tile([P, P, CH], f16)
                nc.vector.tensor_tensor(
                    out=s_t[:], in0=iota_t[:],
                    in1=lrow_t[:, g, None, :].broadcast_to([P, P, CH]),
                    op=mybir.AluOpType.is_equal)
                nc.vector.tensor_tensor(
                    out=s_t[:], in0=s_t[:],
                    in1=sval_t[:, g, None, :].broadcast_to([P, P, CH]),
                    op=mybir.AluOpType.mult)
                acc = accp.tile([P, D], f32)
                for j in range(CH):
                    rhs = (gtE[:, gi * CE + j, :] if j < CE
                           else gtO[:, gi * CO + j - CE, :])
                    nc.tensor.matmul(acc[:], lhsT=s_t[:, :, j],
                                     rhs=rhs,
                                     start=(j == 0), stop=(j == CH - 1))
                if pend is not None:
                    tail(*pend)
                pend = (g, acc)
        tail(*pend)
    return y_d


_PROFILE = False
_TRACE_DIR = None
_LAST_RESULT = None


def kernel(x, W, edge_row, edge_col, edge_vals):
    global _LAST_RESULT
    from concourse import bacc
    from concourse.bass_utils import run_bass_kernel_spmd

    shared, core_inputs, assembly, NG = _host_prep(
        x, W, edge_row, edge_col, edge_vals)

    nc = bacc.Bacc(num_swdge_queues=4)
    _build_program(nc, NG)
    nc.compile()

    in_maps = [dict(shared, **ci) for ci in core_inputs]
    res = run_bass_kernel_spmd(nc, in_maps, core_ids=list(range(NCORES)),
                               trace=_PROFILE, tmpdir=_TRACE_DIR)
    _LAST_RESULT = res

    out = np.zeros((B, N, F), np.float32)
    for c in range(NCORES):
        yc = res.results[c]["y"].astype(np.float32).reshape(-1, P, B, P)
        bases, nrows = assembly[c]
        for g, (base, nr) in enumerate(zip(bases, nrows)):
            out[:, base:base + nr, :] += yc[g, :, :, :nr].transpose(1, 2, 0)
    return out

